# revision 17
# baseline (speedup 1.0000x reference)
"""Trainium2 Bass kernel for nn_MultiHeadQuantileNBEATS.

Reference computation (per batch row b):
  feats = x @ W_bb + b_bb                                   [D]
  h1[q] = relu(feats @ W1[q] + b1[q])                       [QF, H1]
  h2[q] = relu(h1[q] @ W2[q] + b2[q])                       [QF, H2]
  o3[q] = h2[q] @ W3[q] + b3[q]                             [QF, HOR]
  sq    = sort(o3 over q)  (per (b, hor))                   [HOR, QF]
  out[b, h, t] = sort_t(interp(sq[b, h, :], q[b, t]))       [HOR, QT]

Device algorithm notes:
  * Pure data parallel over 8 cores (batch sharded, weights replicated).
  * Head math is feature major ([feature, batch]); x arrives pre-transposed
    from the host so no on-chip input transpose is needed.
  * All matmuls are true fp32: the final interp is a convex combination
    whose result can be ~1e-3 while operands are ~0.5, so any reduced
    precision on the value path (fp32r/tf32 matmuls, low-precision storage
    of sorted values or coefficients) is amplified ~500x by the
    scale-relative error metric and fails the 2e-2 gate.  Only the final
    output store is bf16 (rounding the result itself is a plain <=0.4%
    relative error).
  * The final sort over QT is eliminated: the interpolant is monotone in the
    query level, so using host-sorted q yields already-sorted outputs.
  * Interpolation coefficients a_i(q[b,t]) are precomputed on the host in the
    transposed [(s,i), t] group layout, so the device only transposes the
    sorted head outputs (PE is_transpose path, 4 groups batched per PSUM
    bank with a single evacuation).
  * The block-diagonal coefficient matrix A is built 4 groups per DVE/Pool op
    (broadcast * 0/1 mask).
  * PSUM evacuations are spread across Act (bias/relu fused), DVE and Pool
    (gpsimd); result tiles round-robin across all three.
  * Per-core output is written feature-major [HOR, B_core, QT]; the host
    transposes to [B, HOR, QT] when gathering.
"""

import dataclasses
from contextlib import ExitStack

import numpy as np

import concourse.bass as bass
import concourse.mybir as mybir
import concourse.tile as tile
from concourse import bass_utils
from concourse.bass import ts
from concourse.masks import make_identity

F32 = mybir.dt.float32
BF16 = mybir.dt.bfloat16

B, T, D = 8192, 512, 512
H1, H2, HOR = 256, 128, 96
QF, QT = 7, 32
NCORES = 8
BC = B // NCORES  # batch per core
SUB = 512         # samples per super-tile
GRP = 16          # samples per interp group
GB = 4            # interp groups batched per PSUM bank / A tile
QUANTILE_LEVELS = np.array(
    [0.025, 0.1, 0.25, 0.5, 0.75, 0.9, 0.975], dtype=np.float32
)

# optimal 16-CE sorting network for 7 elements (ascending), disjoint layers
SORT7_LAYERS = [
    [(1, 2), (3, 4), (5, 6)],
    [(0, 2), (3, 5), (4, 6)],
    [(0, 1), (4, 5), (2, 6)],
    [(0, 4), (1, 5)],
    [(0, 3), (2, 5)],
    [(1, 3), (2, 4)],
    [(2, 3)],
]


def _view(ap, free_dims, extra_offset):
    """Rebuild an AP keeping its partition dim, with custom free-dim lattice."""
    dims = [tuple(ap.ap[0])] + [tuple(d) for d in free_dims]
    return dataclasses.replace(ap, ap=tuple(dims), offset=ap.offset + extra_offset)


# ---------------------------------------------------------------------------
# host-side constants / input prep
# ---------------------------------------------------------------------------

def _host_constants(b_bb, b1, b2, b3):
    # bias_all [128, 32]: packed per-partition bias columns
    bias = np.zeros((128, 32), dtype=np.float32)
    for dc in range(D // 128):
        bias[:, dc] = b_bb[128 * dc : 128 * (dc + 1)]
    for qh in range(QF):
        for mc in range(H1 // 128):
            bias[:, 4 + 2 * qh + mc] = b1[qh, 128 * mc : 128 * (mc + 1)]
        bias[:, 18 + qh] = b2[qh]
        bias[:96, 25 + qh] = b3[qh]
    # M112 [112, 512]: block-diagonal 0/1 mask over (sample, coeff) x (sample, t)
    m112 = np.zeros((112, 512), dtype=np.float32)
    for i in range(QF):
        for s in range(GRP):
            m112[GRP * i + s, QT * s : QT * s + QT] = 1.0
    return bias, m112


def _host_coeffs(q_sorted):
    """Interp coefficients for sorted q, in transposed group layout.

    Returns aT packed as [7*GRP, (B//GRP)*QT] fp32 where
    aT[7*sl + i, QT*g + t] = a_i(q_sorted[GRP*g + sl, t]).
    """
    ql = QUANTILE_LEVELS
    v = q_sorted  # [B, QT] fp32
    hi = np.clip(np.searchsorted(ql, v, side="left"), 1, QF - 1)
    lo = hi - 1
    w = ((v - ql[lo]) / (ql[hi] - ql[lo] + np.float32(1e-8))).astype(np.float32)
    a = np.zeros((v.shape[0], QT, QF), dtype=np.float32)
    np.put_along_axis(a, lo[:, :, None], (1.0 - w)[:, :, None], axis=2)
    np.put_along_axis(a, hi[:, :, None], w[:, :, None], axis=2)
    low_mask = v <= ql[0]
    high_mask = v >= ql[-1]
    a[low_mask] = 0.0
    a[high_mask] = 0.0
    a[..., 0] += low_mask.astype(np.float32)
    a[..., QF - 1] += high_mask.astype(np.float32)
    ngrp = v.shape[0] // GRP
    # [B, QT, QF] -> [ngrp, GRP(sl), QT, QF] -> [QF, GRP, ngrp, QT]
    # partition p = GRP*i + sl  (i-major keeps device-side writes contiguous)
    aT = a.reshape(ngrp, GRP, QT, QF).transpose(3, 1, 0, 2)
    return np.ascontiguousarray(aT.reshape(QF * GRP, ngrp * QT))


# ---------------------------------------------------------------------------
# device kernel
# ---------------------------------------------------------------------------

def _emit(ctx: ExitStack, tc: tile.TileContext, ins, outs, bc=BC):
    nc = tc.nc
    xT_d, aT_d, wbb_d, w1_d, w2_d, w3_d, bias_d, m112_d = ins
    (r_d,) = outs
    n_sub = bc // SUB
    ngrp = SUB // GRP

    cpool = ctx.enter_context(tc.tile_pool(name="cpool", bufs=1))
    wpool = ctx.enter_context(tc.tile_pool(name="wpool", bufs=1))
    atpool = ctx.enter_context(tc.tile_pool(name="atpool", bufs=2))
    xTpool = ctx.enter_context(tc.tile_pool(name="xTpool", bufs=2))
    fpool = ctx.enter_context(tc.tile_pool(name="fpool", bufs=2))
    h1pool = ctx.enter_context(tc.tile_pool(name="h1pool", bufs=2))
    h2pool = ctx.enter_context(tc.tile_pool(name="h2pool", bufs=2))
    o3pool = ctx.enter_context(tc.tile_pool(name="o3pool", bufs=12))
    sqgpool = ctx.enter_context(tc.tile_pool(name="sqgpool", bufs=1))
    sqTpool = ctx.enter_context(tc.tile_pool(name="sqTpool", bufs=2))
    apool = ctx.enter_context(tc.tile_pool(name="apool", bufs=2))
    rpool = ctx.enter_context(tc.tile_pool(name="rpool", bufs=4))
    tpsum = ctx.enter_context(tc.tile_pool(name="tpsum", bufs=2, space="PSUM"))
    hpsum = ctx.enter_context(tc.tile_pool(name="hpsum", bufs=2, space="PSUM"))
    rpsum = ctx.enter_context(tc.tile_pool(name="rpsum", bufs=3, space="PSUM"))

    # --- constants ---
    ident = cpool.tile([128, 128], F32)
    make_identity(nc, ident[:])
    bias_sb = cpool.tile([128, 32], F32)
    nc.sync.dma_start(bias_sb[:], bias_d)
    m112 = cpool.tile([112, 512], F32)
    nc.sync.dma_start(m112[:], m112_d)

    # PE warm-up: absorb the GPSIMD (identity build) clock into PE's vector
    # clock so later transposes don't accumulate a third sync wait.
    warm_ps = tpsum.tile([128, 96], F32, tag="warm", bufs=1)
    nc.tensor.matmul(warm_ps[:], lhsT=ident[:], rhs=ident[:, :96],
                     start=True, stop=True)

    # --- weights (persist across super-tiles) ---
    wbb_sb = []
    for tci in range(T // 128):
        w = wpool.tile([128, D], F32, name=f"wbb{tci}")
        nc.sync.dma_start(w[:], wbb_d[ts(tci, 128), :])
        wbb_sb.append(w)
    w1_sb = []
    for qh in range(QF):
        row = []
        for dc in range(D // 128):
            w = wpool.tile([128, H1], F32, name=f"w1_{qh}_{dc}")
            nc.sync.dma_start(w[:], w1_d[qh, ts(dc, 128), :])
            row.append(w)
        w1_sb.append(row)
    w2_sb = []
    for qh in range(QF):
        row = []
        for mc in range(H1 // 128):
            w = wpool.tile([128, H2], F32, name=f"w2_{qh}_{mc}")
            nc.sync.dma_start(w[:], w2_d[qh, ts(mc, 128), :])
            row.append(w)
        w2_sb.append(row)
    w3_sb = []
    for qh in range(QF):
        w = wpool.tile([128, HOR], F32, name=f"w3_{qh}")
        nc.sync.dma_start(w[:], w3_d[qh])
        w3_sb.append(w)

    # =====================================================================
    # heads + sort + interpolation, per super-tile of 512 samples
    # =====================================================================
    for st in range(n_sub):
        # ---- loads: transposed x chunks + transposed coefficient block ----
        xT = []
        for tci in range(4):
            xt = xTpool.tile([128, SUB], F32, name=f"xT{st}_{tci}", tag=f"xT{tci}")
            nc.sync.dma_start(xt[:], xT_d[ts(tci, 128), ts(st, SUB)])
            xT.append(xt)
        aT = atpool.tile([112, ngrp * QT], F32, name=f"aT{st}", tag="aT")
        nc.sync.dma_start(aT[:], aT_d[:, ts(st, ngrp * QT)])

        # ---- backbone: featsT[dc] [128, SUB] ----
        featsT = []
        for dc in range(4):
            ps = hpsum.tile([128, SUB], F32, tag="hps")
            for tci in range(4):
                nc.tensor.matmul(
                    ps[:],
                    lhsT=wbb_sb[tci][:, ts(dc, 128)],
                    rhs=xT[tci][:],
                    start=(tci == 0),
                    stop=(tci == 3),
                )
            ft = fpool.tile([128, SUB], F32, name=f"feats{st}_{dc}", tag=f"ft{dc}")
            nc.scalar.activation(
                ft[:], ps[:], mybir.ActivationFunctionType.Identity,
                bias=bias_sb[:, dc : dc + 1], scale=1.0,
            )
            featsT.append(ft)

        # ---- W1: h1T[q][mc] [128, SUB] ----
        h1T = [[None] * 2 for _ in range(QF)]
        for qh in range(QF):
            for mc in range(2):
                ps = hpsum.tile([128, SUB], F32, tag="hps")
                for dc in range(4):
                    nc.tensor.matmul(
                        ps[:],
                        lhsT=w1_sb[qh][dc][:, ts(mc, 128)],
                        rhs=featsT[dc][:],
                        start=(dc == 0),
                        stop=(dc == 3),
                    )
                h1 = h1pool.tile([128, SUB], F32, name=f"h1_{st}_{qh}_{mc}",
                                  tag=f"h1_{qh}_{mc}", bufs=1)
                nc.scalar.activation(
                    h1[:], ps[:], mybir.ActivationFunctionType.Relu,
                    bias=bias_sb[:, 4 + 2 * qh + mc : 5 + 2 * qh + mc], scale=1.0,
                )
                h1T[qh][mc] = h1

        # ---- W2: h2T[q] [128, SUB] ----
        h2T = [None] * QF
        for qh in range(QF):
            ps = hpsum.tile([128, SUB], F32, tag="hps")
            for mc in range(2):
                nc.tensor.matmul(
                    ps[:], lhsT=w2_sb[qh][mc][:], rhs=h1T[qh][mc][:],
                    start=(mc == 0), stop=(mc == 1),
                )
            h2 = h2pool.tile([128, SUB], F32, name=f"h2_{st}_{qh}", tag=f"h2_{qh}",
                                  bufs=1)
            nc.scalar.activation(
                h2[:], ps[:], mybir.ActivationFunctionType.Relu,
                bias=bias_sb[:, 18 + qh : 19 + qh], scale=1.0,
            )
            h2T[qh] = h2

        # ---- W3: o3[q] [96, SUB] (bias added on DVE) ----
        o3 = [None] * QF
        for qh in range(QF):
            ps = hpsum.tile([HOR, SUB], F32, tag="hps")
            nc.tensor.matmul(
                ps[:], lhsT=w3_sb[qh][:, :], rhs=h2T[qh][:],
                start=True, stop=True,
            )
            o = o3pool.tile([HOR, SUB], F32, name=f"o3_{st}_{qh}", tag="sortt")
            nc.scalar.activation(
                o[:], ps[:], mybir.ActivationFunctionType.Identity,
                bias=bias_sb[:HOR, 25 + qh : 26 + qh], scale=1.0,
            )
            o3[qh] = o

        # ---- sort the 7 head outputs elementwise on DVE with flat
        # contiguous APs (3-D views run the DVE slow path); final values
        # written interleaved into SQG (free idx = 112*g + 16*i + s)
        sqg = sqgpool.tile([HOR, ngrp * 112], F32, name=f"sqg{st}", tag="sqg")
        last_touch = {}
        for li, layer in enumerate(SORT7_LAYERS):
            for (a, b) in layer:
                last_touch[a] = (li, a, b)
                last_touch[b] = (li, a, b)
        cur = {k: o3[k] for k in range(QF)}

        def sqg_slot(j):
            return _view(sqg[:], [(112, ngrp), (1, GRP)], GRP * j)

        ce_idx = 0
        for li, layer in enumerate(SORT7_LAYERS):
            for (a, b) in layer:
                a_final = last_touch[a] == (li, a, b)
                b_final = last_touch[b] == (li, a, b)
                ia = cur[a][:]
                ib = cur[b][:]
                if a_final:
                    oa = sqg_slot(a)
                    ia = ia.rearrange("p (g s) -> p g s", g=ngrp)
                    ib_a = cur[b][:].rearrange("p (g s) -> p g s", g=ngrp)
                else:
                    ta = o3pool.tile([HOR, SUB], F32, name=f"s{st}_{ce_idx}a",
                                     tag="sortt")
                    oa = ta[:]
                    ib_a = ib
                if b_final:
                    ob = sqg_slot(b)
                    ia_b = cur[a][:].rearrange("p (g s) -> p g s", g=ngrp)
                    ib_b = cur[b][:].rearrange("p (g s) -> p g s", g=ngrp)
                else:
                    tb = o3pool.tile([HOR, SUB], F32, name=f"s{st}_{ce_idx}b",
                                     tag="sortt")
                    ob = tb[:]
                    ia_b = cur[a][:]
                    ib_b = cur[b][:]
                nc.vector.tensor_tensor(oa, ia, ib_a, op=mybir.AluOpType.min)
                nc.vector.tensor_tensor(ob, ia_b, ib_b, op=mybir.AluOpType.max)
                if not a_final:
                    cur[a] = ta
                if not b_final:
                    cur[b] = tb
                ce_idx += 1

        # ---- per 4-group batch: transposes into one PSUM bank + single
        # evacuation, batched A build, then per-group interp matmul ----
        for bi in range(ngrp // GB):
            ps_t = tpsum.tile([112, GB * HOR], F32, tag="tps")
            for j in range(GB):
                g = GB * bi + j
                nc.tensor.transpose(
                    ps_t[:, ts(j, HOR)], sqg[:, 112 * g : 112 * (g + 1)],
                    ident[:HOR, :HOR],
                )
            sqa = sqTpool.tile([112, GB * HOR], F32, tag="sqa")
            nc.vector.tensor_copy(sqa[:], ps_t[:])

            # A [112, GB*512] = broadcast(aT 4-group block) * M112
            A = apool.tile([112, GB * 512], F32, tag="A")
            av = aT[:, GB * QT * bi : GB * QT * (bi + 1)].rearrange(
                "p (j t) -> p j t", j=GB
            ).unsqueeze(2).broadcast_to((112, GB, GRP, QT))
            mv = m112[:].rearrange("p (s t) -> p s t", s=GRP).unsqueeze(
                1
            ).broadcast_to((112, GB, GRP, QT))
            Av = A[:].rearrange("p (j s t) -> p j s t", j=GB, s=GRP)
            nc.vector.tensor_tensor(Av, av, mv, op=mybir.AluOpType.mult)

            for j in range(GB):
                g = GB * bi + j
                gg = st * ngrp + g  # global group index
                rps = rpsum.tile([HOR, 512], F32, tag="rps")
                nc.tensor.matmul(
                    rps[:], lhsT=sqa[:, ts(j, HOR)],
                    rhs=A[:, ts(j, 512)], start=True, stop=True,
                )
                r_sb = rpool.tile([HOR, 512], BF16, tag="rsb")
                if g % 2 == 0:
                    nc.scalar.copy(r_sb[:], rps[:])
                else:
                    nc.vector.tensor_copy(r_sb[:], rps[:])
                nc.scalar.dma_start(
                    r_d[:, GRP * gg : GRP * (gg + 1), :],
                    r_sb[:].rearrange("p (s t) -> p s t", s=GRP),
                )


# Per-instruction-type sync-wait slot capacity in the walrus ISA descriptors.
_WAIT_CAPACITY = {}  # default: every type gets a single wait slot
_DRAIN_CAPACITY = {
    "EngineType.SP": 1,
    "EngineType.PE": 1,
}


def _split_waits(nc):
    """Some walrus ISA descriptors (LDWEIGHTS, DMA) have too few sync-wait
    slots for the waits Tile emits.  Move surplus waits of overflowing
    instructions onto drains inserted right before them on the same queue."""
    for fn in nc.m.functions:
        for blk in fn.blocks:
            insts = list(blk.instructions)
            out = []
            changed = False
            for ins in insts:
                si = ins.sync_info
                cap = _WAIT_CAPACITY.get(type(ins).__name__, 1)
                if si is not None and si.on_wait and len(si.on_wait) > cap:
                    waits = list(si.on_wait)
                    surplus = waits[:-cap]
                    dcap = _DRAIN_CAPACITY.get(str(ins.engine), 1)
                    di = 0
                    while surplus:
                        chunk, surplus = surplus[:dcap], surplus[dcap:]
                        out.append(
                            mybir.InstDrain(
                                name=f"{ins.name}-wfence{di}",
                                engine=ins.engine,
                                ins=[],
                                outs=[],
                                sync_info=mybir.SyncInfo(
                                    on_wait=chunk, on_update=[]
                                ),
                            )
                        )
                        di += 1
                    si.on_wait = waits[-cap:]
                    changed = True
                out.append(ins)
            if changed:
                blk.instructions = out


def build_module(bc=BC):
    nc = bass.Bass("TRN2", target_bir_lowering=False, debug=False)
    xT_d = nc.dram_tensor("xT", [T, bc], F32, kind="ExternalInput").ap()
    aT_d = nc.dram_tensor("aT", [GRP * QF, (bc // GRP) * QT], F32,
                          kind="ExternalInput").ap()
    wbb_d = nc.dram_tensor("W_bb", [T, D], F32, kind="ExternalInput").ap()
    w1_d = nc.dram_tensor("W1", [QF, D, H1], F32, kind="ExternalInput").ap()
    w2_d = nc.dram_tensor("W2", [QF, H1, H2], F32, kind="ExternalInput").ap()
    w3_d = nc.dram_tensor("W3", [QF, H2, HOR], F32, kind="ExternalInput").ap()
    bias_d = nc.dram_tensor("bias_all", [128, 32], F32, kind="ExternalInput").ap()
    m112_d = nc.dram_tensor("m112", [112, 512], F32, kind="ExternalInput").ap()
    r_d = nc.dram_tensor("r_out", [HOR, bc, QT], BF16, kind="ExternalOutput").ap()

    with tile.TileContext(nc) as tc:
        with ExitStack() as ctx:
            _emit(ctx, tc, (xT_d, aT_d, wbb_d, w1_d, w2_d, w3_d, bias_d, m112_d),
                  (r_d,), bc=bc)
    _split_waits(nc)
    return nc


_NC_CACHE = {}
LAST_RESULTS = None


def kernel(**inputs) -> np.ndarray:
    global LAST_RESULTS
    x = np.asarray(inputs["x"], dtype=np.float32)
    q = np.asarray(inputs["q"], dtype=np.float32)
    w_bb = np.ascontiguousarray(np.asarray(inputs["W_bb"], dtype=np.float32))
    w1 = np.ascontiguousarray(np.asarray(inputs["W1"], dtype=np.float32))
    w2 = np.ascontiguousarray(np.asarray(inputs["W2"], dtype=np.float32))
    w3 = np.ascontiguousarray(np.asarray(inputs["W3"], dtype=np.float32))
    bias, m112 = _host_constants(
        np.asarray(inputs["b_bb"], dtype=np.float32),
        np.asarray(inputs["b1"], dtype=np.float32),
        np.asarray(inputs["b2"], dtype=np.float32),
        np.asarray(inputs["b3"], dtype=np.float32),
    )
    q_sorted = np.sort(q, axis=1)

    if BC not in _NC_CACHE:
        _NC_CACHE[BC] = build_module(BC)
    nc = _NC_CACHE[BC]

    in_maps = []
    for c in range(NCORES):
        sl = slice(BC * c, BC * (c + 1))
        in_maps.append(
            {
                "xT": np.ascontiguousarray(x[sl].T),
                "aT": _host_coeffs(q_sorted[sl]),
                "W_bb": w_bb,
                "W1": w1,
                "W2": w2,
                "W3": w3,
                "bias_all": bias,
                "m112": m112,
            }
        )

    res = bass_utils.run_bass_kernel_spmd(nc, in_maps, core_ids=list(range(NCORES)))
    LAST_RESULTS = res
    out = np.empty((B, HOR, QT), dtype=np.float32)
    for c in range(NCORES):
        out[BC * c : BC * (c + 1)] = np.transpose(
            res.results[c]["r_out"].astype(np.float32), (1, 0, 2)
        )
    return out


# revision 19
# speedup vs baseline: 1.2045x; 1.2045x over previous
"""Trainium2 Bass kernel for nn_MultiHeadQuantileNBEATS.

Reference computation (per batch row b):
  feats = x @ W_bb + b_bb                                   [D]
  h1[q] = relu(feats @ W1[q] + b1[q])                       [QF, H1]
  h2[q] = relu(h1[q] @ W2[q] + b2[q])                       [QF, H2]
  o3[q] = h2[q] @ W3[q] + b3[q]                             [QF, HOR]
  sq    = sort(o3 over q)  (per (b, hor))                   [HOR, QF]
  out[b, h, t] = sort_t(interp(sq[b, h, :], q[b, t]))       [HOR, QT]

Device algorithm notes:
  * Pure data parallel over 8 cores (batch sharded, weights replicated).
  * Head math is feature major ([feature, batch]); x arrives pre-transposed
    from the host so no on-chip input transpose is needed.
  * All matmuls are true fp32: the final interp is a convex combination
    whose result can be ~1e-3 while operands are ~0.5, so any reduced
    precision on the value path (fp32r/tf32 matmuls, low-precision storage
    of sorted values or coefficients) is amplified ~500x by the
    scale-relative error metric and fails the 2e-2 gate.  Only the final
    output store is bf16 (rounding the result itself is a plain <=0.4%
    relative error).
  * The final sort over QT is eliminated: the interpolant is monotone in the
    query level, so using host-sorted q yields already-sorted outputs.
  * Interpolation coefficients a_i(q[b,t]) are precomputed on the host in the
    transposed [(s,i), t] group layout, so the device only transposes the
    sorted head outputs (PE is_transpose path, 4 groups batched per PSUM
    bank with a single evacuation).
  * The block-diagonal coefficient matrix A is built 4 groups per DVE/Pool op
    (broadcast * 0/1 mask).
  * PSUM evacuations are spread across Act (bias/relu fused), DVE and Pool
    (gpsimd); result tiles round-robin across all three.
  * Per-core output is written feature-major [HOR, B_core, QT]; the host
    transposes to [B, HOR, QT] when gathering.
"""

import dataclasses
from contextlib import ExitStack

import numpy as np

import concourse.bass as bass
import concourse.mybir as mybir
import concourse.tile as tile
from concourse import bass_utils
from concourse.bass import ts
from concourse.masks import make_identity

F32 = mybir.dt.float32
BF16 = mybir.dt.bfloat16

B, T, D = 8192, 512, 512
H1, H2, HOR = 256, 128, 96
QF, QT = 7, 32
NCORES = 8
BC = B // NCORES  # batch per core
SUB = 512         # samples per super-tile
GRP = 16          # samples per interp group
GB = 4            # interp groups batched per PSUM bank / A tile
QUANTILE_LEVELS = np.array(
    [0.025, 0.1, 0.25, 0.5, 0.75, 0.9, 0.975], dtype=np.float32
)

# optimal 16-CE sorting network for 7 elements (ascending), disjoint layers
SORT7_LAYERS = [
    [(1, 2), (3, 4), (5, 6)],
    [(0, 2), (3, 5), (4, 6)],
    [(0, 1), (4, 5), (2, 6)],
    [(0, 4), (1, 5)],
    [(0, 3), (2, 5)],
    [(1, 3), (2, 4)],
    [(2, 3)],
]


def _view(ap, free_dims, extra_offset):
    """Rebuild an AP keeping its partition dim, with custom free-dim lattice."""
    dims = [tuple(ap.ap[0])] + [tuple(d) for d in free_dims]
    return dataclasses.replace(ap, ap=tuple(dims), offset=ap.offset + extra_offset)


# ---------------------------------------------------------------------------
# host-side constants / input prep
# ---------------------------------------------------------------------------

def _host_constants(b_eff, b2, b3):
    # bias_all [128, 32]: packed per-partition bias columns
    bias = np.zeros((128, 32), dtype=np.float32)
    for qh in range(QF):
        for mc in range(H1 // 128):
            bias[:, 4 + 2 * qh + mc] = b_eff[qh, 128 * mc : 128 * (mc + 1)]
        bias[:, 18 + qh] = b2[qh]
        bias[:96, 25 + qh] = b3[qh]
    # M112 [112, 512]: block-diagonal 0/1 mask over (sample, coeff) x (sample, t)
    m112 = np.zeros((112, 512), dtype=np.float32)
    for i in range(QF):
        for s in range(GRP):
            m112[GRP * i + s, QT * s : QT * s + QT] = 1.0
    return bias, m112


def _host_coeffs(q_sorted):
    """Interp coefficients for sorted q, in transposed group layout.

    Returns aT packed as [7*GRP, (B//GRP)*QT] fp32 where
    aT[7*sl + i, QT*g + t] = a_i(q_sorted[GRP*g + sl, t]).
    """
    ql = QUANTILE_LEVELS
    v = q_sorted  # [B, QT] fp32
    hi = np.clip(np.searchsorted(ql, v, side="left"), 1, QF - 1)
    lo = hi - 1
    w = ((v - ql[lo]) / (ql[hi] - ql[lo] + np.float32(1e-8))).astype(np.float32)
    a = np.zeros((v.shape[0], QT, QF), dtype=np.float32)
    np.put_along_axis(a, lo[:, :, None], (1.0 - w)[:, :, None], axis=2)
    np.put_along_axis(a, hi[:, :, None], w[:, :, None], axis=2)
    low_mask = v <= ql[0]
    high_mask = v >= ql[-1]
    a[low_mask] = 0.0
    a[high_mask] = 0.0
    a[..., 0] += low_mask.astype(np.float32)
    a[..., QF - 1] += high_mask.astype(np.float32)
    ngrp = v.shape[0] // GRP
    # [B, QT, QF] -> [ngrp, GRP(sl), QT, QF] -> [QF, GRP, ngrp, QT]
    # partition p = GRP*i + sl  (i-major keeps device-side writes contiguous)
    aT = a.reshape(ngrp, GRP, QT, QF).transpose(3, 1, 0, 2)
    return np.ascontiguousarray(aT.reshape(QF * GRP, ngrp * QT))


# ---------------------------------------------------------------------------
# device kernel
# ---------------------------------------------------------------------------

def _emit(ctx: ExitStack, tc: tile.TileContext, ins, outs, bc=BC):
    nc = tc.nc
    xT_d, aT_d, w1_d, w2_d, w3_d, bias_d, m112_d = ins
    (r_d,) = outs
    n_sub = bc // SUB
    ngrp = SUB // GRP

    cpool = ctx.enter_context(tc.tile_pool(name="cpool", bufs=1))
    wpool = ctx.enter_context(tc.tile_pool(name="wpool", bufs=1))
    atpool = ctx.enter_context(tc.tile_pool(name="atpool", bufs=2))
    xTpool = ctx.enter_context(tc.tile_pool(name="xTpool", bufs=2))
    h1pool = ctx.enter_context(tc.tile_pool(name="h1pool", bufs=2))
    h2pool = ctx.enter_context(tc.tile_pool(name="h2pool", bufs=2))
    o3pool = ctx.enter_context(tc.tile_pool(name="o3pool", bufs=12))
    sqgpool = ctx.enter_context(tc.tile_pool(name="sqgpool", bufs=1))
    sqTpool = ctx.enter_context(tc.tile_pool(name="sqTpool", bufs=2))
    apool = ctx.enter_context(tc.tile_pool(name="apool", bufs=2))
    rpool = ctx.enter_context(tc.tile_pool(name="rpool", bufs=4))
    tpsum = ctx.enter_context(tc.tile_pool(name="tpsum", bufs=3, space="PSUM"))
    hpsum = ctx.enter_context(tc.tile_pool(name="hpsum", bufs=2, space="PSUM"))
    rpsum = ctx.enter_context(tc.tile_pool(name="rpsum", bufs=3, space="PSUM"))

    # --- constants ---
    ident = cpool.tile([128, 128], F32)
    make_identity(nc, ident[:])
    bias_sb = cpool.tile([128, 32], F32)
    nc.sync.dma_start(bias_sb[:], bias_d)
    m112 = cpool.tile([112, 512], F32)
    nc.sync.dma_start(m112[:], m112_d)

    # ---- super-tile 0 input loads FIRST so the PE can start as soon as the
    # first W_eff chunks land (weights queue behind on the same DMA queue) ----
    xT_st = []
    aT_st = []
    for st in range(n_sub):
        xT_st.append([None] * 4)
        aT_st.append(None)
    for tci in range(4):
        xt = xTpool.tile([128, SUB], F32, name=f"xT0_{tci}", tag=f"xT{tci}")
        nc.sync.dma_start(xt[:], xT_d[ts(tci, 128), ts(0, SUB)])
        xT_st[0][tci] = xt

    # --- weights (persist across super-tiles); W_eff = W_bb @ W1 host-fused ---
    w1_sb = []
    for qh in range(QF):
        row = []
        for dc in range(D // 128):
            w = wpool.tile([128, H1], F32, name=f"w1_{qh}_{dc}")
            nc.sync.dma_start(w[:], w1_d[qh, ts(dc, 128), :])
            row.append(w)
        w1_sb.append(row)
    aT0 = atpool.tile([112, ngrp * QT], F32, name="aT0", tag="aT")
    nc.sync.dma_start(aT0[:], aT_d[:, ts(0, ngrp * QT)])
    aT_st[0] = aT0
    w2_sb = []
    for qh in range(QF):
        row = []
        for mc in range(H1 // 128):
            w = wpool.tile([128, H2], F32, name=f"w2_{qh}_{mc}")
            nc.sync.dma_start(w[:], w2_d[qh, ts(mc, 128), :])
            row.append(w)
        w2_sb.append(row)
    w3_sb = []
    for qh in range(QF):
        w = wpool.tile([128, HOR], F32, name=f"w3_{qh}")
        nc.sync.dma_start(w[:], w3_d[qh])
        w3_sb.append(w)

    # =====================================================================
    # heads + sort + interpolation, per super-tile of 512 samples
    # =====================================================================
    for st in range(n_sub):
        # ---- loads: transposed x chunks + transposed coefficient block ----
        if xT_st[st][0] is None:
            for tci in range(4):
                xt = xTpool.tile([128, SUB], F32, name=f"xT{st}_{tci}",
                                 tag=f"xT{tci}")
                nc.sync.dma_start(xt[:], xT_d[ts(tci, 128), ts(st, SUB)])
                xT_st[st][tci] = xt
            a_t = atpool.tile([112, ngrp * QT], F32, name=f"aT{st}", tag="aT")
            nc.sync.dma_start(a_t[:], aT_d[:, ts(st, ngrp * QT)])
            aT_st[st] = a_t
        xT = xT_st[st]
        aT = aT_st[st]

        # ---- W_eff: h1T[q][mc] [128, SUB] straight from xT ----
        h1T = [[None] * 2 for _ in range(QF)]
        for qh in range(QF):
            for mc in range(2):
                ps = hpsum.tile([128, SUB], F32, tag="hps")
                for dc in range(4):
                    nc.tensor.matmul(
                        ps[:],
                        lhsT=w1_sb[qh][dc][:, ts(mc, 128)],
                        rhs=xT[dc][:],
                        start=(dc == 0),
                        stop=(dc == 3),
                    )
                h1 = h1pool.tile([128, SUB], F32, name=f"h1_{st}_{qh}_{mc}",
                                  tag=f"h1_{qh}_{mc}", bufs=1)
                nc.scalar.activation(
                    h1[:], ps[:], mybir.ActivationFunctionType.Relu,
                    bias=bias_sb[:, 4 + 2 * qh + mc : 5 + 2 * qh + mc], scale=1.0,
                )
                h1T[qh][mc] = h1

        # ---- W2: h2T[q] [128, SUB] ----
        h2T = [None] * QF
        for qh in range(QF):
            ps = hpsum.tile([128, SUB], F32, tag="hps")
            for mc in range(2):
                nc.tensor.matmul(
                    ps[:], lhsT=w2_sb[qh][mc][:], rhs=h1T[qh][mc][:],
                    start=(mc == 0), stop=(mc == 1),
                )
            h2 = h2pool.tile([128, SUB], F32, name=f"h2_{st}_{qh}", tag=f"h2_{qh}",
                                  bufs=1)
            nc.scalar.activation(
                h2[:], ps[:], mybir.ActivationFunctionType.Relu,
                bias=bias_sb[:, 18 + qh : 19 + qh], scale=1.0,
            )
            h2T[qh] = h2

        # ---- W3: o3[q] [96, SUB] (bias added on DVE) ----
        o3 = [None] * QF
        for qh in range(QF):
            ps = hpsum.tile([HOR, SUB], F32, tag="hps")
            nc.tensor.matmul(
                ps[:], lhsT=w3_sb[qh][:, :], rhs=h2T[qh][:],
                start=True, stop=True,
            )
            o = o3pool.tile([HOR, SUB], F32, name=f"o3_{st}_{qh}", tag="sortt")
            nc.scalar.activation(
                o[:], ps[:], mybir.ActivationFunctionType.Identity,
                bias=bias_sb[:HOR, 25 + qh : 26 + qh], scale=1.0,
            )
            o3[qh] = o

        # ---- sort the 7 head outputs elementwise on DVE with flat
        # contiguous APs (3-D views run the DVE slow path); final values
        # written interleaved into SQG (free idx = 112*g + 16*i + s)
        sqg = sqgpool.tile([HOR, ngrp * 112], F32, name=f"sqg{st}", tag="sqg")
        last_touch = {}
        for li, layer in enumerate(SORT7_LAYERS):
            for (a, b) in layer:
                last_touch[a] = (li, a, b)
                last_touch[b] = (li, a, b)
        cur = {k: o3[k] for k in range(QF)}

        def sqg_slot(j):
            return _view(sqg[:], [(112, ngrp), (1, GRP)], GRP * j)

        ce_idx = 0
        for li, layer in enumerate(SORT7_LAYERS):
            for (a, b) in layer:
                a_final = last_touch[a] == (li, a, b)
                b_final = last_touch[b] == (li, a, b)
                ia = cur[a][:]
                ib = cur[b][:]
                if a_final:
                    oa = sqg_slot(a)
                    ia = ia.rearrange("p (g s) -> p g s", g=ngrp)
                    ib_a = cur[b][:].rearrange("p (g s) -> p g s", g=ngrp)
                else:
                    ta = o3pool.tile([HOR, SUB], F32, name=f"s{st}_{ce_idx}a",
                                     tag="sortt")
                    oa = ta[:]
                    ib_a = ib
                if b_final:
                    ob = sqg_slot(b)
                    ia_b = cur[a][:].rearrange("p (g s) -> p g s", g=ngrp)
                    ib_b = cur[b][:].rearrange("p (g s) -> p g s", g=ngrp)
                else:
                    tb = o3pool.tile([HOR, SUB], F32, name=f"s{st}_{ce_idx}b",
                                     tag="sortt")
                    ob = tb[:]
                    ia_b = cur[a][:]
                    ib_b = cur[b][:]
                nc.vector.tensor_tensor(oa, ia, ib_a, op=mybir.AluOpType.min)
                nc.vector.tensor_tensor(ob, ia_b, ib_b, op=mybir.AluOpType.max)
                if not a_final:
                    cur[a] = ta
                if not b_final:
                    cur[b] = tb
                ce_idx += 1

        # ---- per 4-group batch: transposes into one PSUM bank + single
        # evacuation, batched A build, then per-group interp matmul ----
        for bi in range(ngrp // GB):
            ps_t = tpsum.tile([112, GB * HOR], F32, tag="tps")
            for j in range(GB):
                g = GB * bi + j
                nc.tensor.transpose(
                    ps_t[:, ts(j, HOR)], sqg[:, 112 * g : 112 * (g + 1)],
                    ident[:HOR, :HOR],
                )
            sqa = sqTpool.tile([112, GB * HOR], F32, tag="sqa")
            nc.vector.tensor_copy(sqa[:], ps_t[:])

            # A [112, GB*512] = broadcast(aT 4-group block) * M112
            A = apool.tile([112, GB * 512], F32, tag="A")
            av = aT[:, GB * QT * bi : GB * QT * (bi + 1)].rearrange(
                "p (j t) -> p j t", j=GB
            ).unsqueeze(2).broadcast_to((112, GB, GRP, QT))
            mv = m112[:].rearrange("p (s t) -> p s t", s=GRP).unsqueeze(
                1
            ).broadcast_to((112, GB, GRP, QT))
            Av = A[:].rearrange("p (j s t) -> p j s t", j=GB, s=GRP)
            nc.vector.tensor_tensor(Av, av, mv, op=mybir.AluOpType.mult)

            for j in range(GB):
                g = GB * bi + j
                gg = st * ngrp + g  # global group index
                rps = rpsum.tile([HOR, 512], F32, tag="rps")
                nc.tensor.matmul(
                    rps[:], lhsT=sqa[:, ts(j, HOR)],
                    rhs=A[:, ts(j, 512)], start=True, stop=True,
                )
                r_sb = rpool.tile([HOR, 512], BF16, tag="rsb")
                if g % 2 == 0:
                    nc.scalar.copy(r_sb[:], rps[:])
                else:
                    nc.vector.tensor_copy(r_sb[:], rps[:])
                nc.scalar.dma_start(
                    r_d[:, GRP * gg : GRP * (gg + 1), :],
                    r_sb[:].rearrange("p (s t) -> p s t", s=GRP),
                )


# Per-instruction-type sync-wait slot capacity in the walrus ISA descriptors.
_WAIT_CAPACITY = {}  # default: every type gets a single wait slot
_DRAIN_CAPACITY = {
    "EngineType.SP": 1,
    "EngineType.PE": 1,
}


def _split_waits(nc):
    """Some walrus ISA descriptors (LDWEIGHTS, DMA) have too few sync-wait
    slots for the waits Tile emits.  Move surplus waits of overflowing
    instructions onto drains inserted right before them on the same queue."""
    for fn in nc.m.functions:
        for blk in fn.blocks:
            insts = list(blk.instructions)
            out = []
            changed = False
            for ins in insts:
                si = ins.sync_info
                cap = _WAIT_CAPACITY.get(type(ins).__name__, 1)
                if si is not None and si.on_wait and len(si.on_wait) > cap:
                    waits = list(si.on_wait)
                    surplus = waits[:-cap]
                    dcap = _DRAIN_CAPACITY.get(str(ins.engine), 1)
                    di = 0
                    while surplus:
                        chunk, surplus = surplus[:dcap], surplus[dcap:]
                        out.append(
                            mybir.InstDrain(
                                name=f"{ins.name}-wfence{di}",
                                engine=ins.engine,
                                ins=[],
                                outs=[],
                                sync_info=mybir.SyncInfo(
                                    on_wait=chunk, on_update=[]
                                ),
                            )
                        )
                        di += 1
                    si.on_wait = waits[-cap:]
                    changed = True
                out.append(ins)
            if changed:
                blk.instructions = out


def build_module(bc=BC):
    nc = bass.Bass("TRN2", target_bir_lowering=False, debug=False)
    xT_d = nc.dram_tensor("xT", [T, bc], F32, kind="ExternalInput").ap()
    aT_d = nc.dram_tensor("aT", [GRP * QF, (bc // GRP) * QT], F32,
                          kind="ExternalInput").ap()
    w1_d = nc.dram_tensor("Weff", [QF, T, H1], F32, kind="ExternalInput").ap()
    w2_d = nc.dram_tensor("W2", [QF, H1, H2], F32, kind="ExternalInput").ap()
    w3_d = nc.dram_tensor("W3", [QF, H2, HOR], F32, kind="ExternalInput").ap()
    bias_d = nc.dram_tensor("bias_all", [128, 32], F32, kind="ExternalInput").ap()
    m112_d = nc.dram_tensor("m112", [112, 512], F32, kind="ExternalInput").ap()
    r_d = nc.dram_tensor("r_out", [HOR, bc, QT], BF16, kind="ExternalOutput").ap()

    with tile.TileContext(nc) as tc:
        with ExitStack() as ctx:
            _emit(ctx, tc, (xT_d, aT_d, w1_d, w2_d, w3_d, bias_d, m112_d),
                  (r_d,), bc=bc)
    _split_waits(nc)
    return nc


_NC_CACHE = {}
LAST_RESULTS = None


def kernel(**inputs) -> np.ndarray:
    global LAST_RESULTS
    x = np.asarray(inputs["x"], dtype=np.float32)
    q = np.asarray(inputs["q"], dtype=np.float32)
    w_bb = np.asarray(inputs["W_bb"], dtype=np.float64)
    w1_64 = np.asarray(inputs["W1"], dtype=np.float64)
    # Fold the (linear, dropout-free) backbone into the first head layer:
    # h1 = relu(x @ (W_bb @ W1[q]) + (b_bb @ W1[q] + b1[q])).
    w_eff = np.ascontiguousarray(
        np.einsum("td,qdk->qtk", w_bb, w1_64).astype(np.float32)
    )
    b_eff = (
        np.asarray(inputs["b_bb"], dtype=np.float64) @ w1_64
        + np.asarray(inputs["b1"], dtype=np.float64)
    ).astype(np.float32)
    w2 = np.ascontiguousarray(np.asarray(inputs["W2"], dtype=np.float32))
    w3 = np.ascontiguousarray(np.asarray(inputs["W3"], dtype=np.float32))
    bias, m112 = _host_constants(
        b_eff,
        np.asarray(inputs["b2"], dtype=np.float32),
        np.asarray(inputs["b3"], dtype=np.float32),
    )
    q_sorted = np.sort(q, axis=1)

    if BC not in _NC_CACHE:
        _NC_CACHE[BC] = build_module(BC)
    nc = _NC_CACHE[BC]

    in_maps = []
    for c in range(NCORES):
        sl = slice(BC * c, BC * (c + 1))
        in_maps.append(
            {
                "xT": np.ascontiguousarray(x[sl].T),
                "aT": _host_coeffs(q_sorted[sl]),
                "Weff": w_eff,
                "W2": w2,
                "W3": w3,
                "bias_all": bias,
                "m112": m112,
            }
        )

    res = bass_utils.run_bass_kernel_spmd(nc, in_maps, core_ids=list(range(NCORES)))
    LAST_RESULTS = res
    out = np.empty((B, HOR, QT), dtype=np.float32)
    for c in range(NCORES):
        out[BC * c : BC * (c + 1)] = np.transpose(
            res.results[c]["r_out"].astype(np.float32), (1, 0, 2)
        )
    return out


# revision 21
# speedup vs baseline: 1.2062x; 1.0014x over previous
"""Trainium2 Bass kernel for nn_MultiHeadQuantileNBEATS.

Reference computation (per batch row b):
  feats = x @ W_bb + b_bb                                   [D]
  h1[q] = relu(feats @ W1[q] + b1[q])                       [QF, H1]
  h2[q] = relu(h1[q] @ W2[q] + b2[q])                       [QF, H2]
  o3[q] = h2[q] @ W3[q] + b3[q]                             [QF, HOR]
  sq    = sort(o3 over q)  (per (b, hor))                   [HOR, QF]
  out[b, h, t] = sort_t(interp(sq[b, h, :], q[b, t]))       [HOR, QT]

Device algorithm notes:
  * Pure data parallel over 8 cores (batch sharded, weights replicated).
  * Head math is feature major ([feature, batch]); x arrives pre-transposed
    from the host so no on-chip input transpose is needed.
  * All matmuls are true fp32: the final interp is a convex combination
    whose result can be ~1e-3 while operands are ~0.5, so any reduced
    precision on the value path (fp32r/tf32 matmuls, low-precision storage
    of sorted values or coefficients) is amplified ~500x by the
    scale-relative error metric and fails the 2e-2 gate.  Only the final
    output store is bf16 (rounding the result itself is a plain <=0.4%
    relative error).
  * The final sort over QT is eliminated: the interpolant is monotone in the
    query level, so using host-sorted q yields already-sorted outputs.
  * Interpolation coefficients a_i(q[b,t]) are precomputed on the host in the
    transposed [(s,i), t] group layout, so the device only transposes the
    sorted head outputs (PE is_transpose path, 4 groups batched per PSUM
    bank with a single evacuation).
  * The block-diagonal coefficient matrix A is built 4 groups per DVE/Pool op
    (broadcast * 0/1 mask).
  * PSUM evacuations are spread across Act (bias/relu fused), DVE and Pool
    (gpsimd); result tiles round-robin across all three.
  * Per-core output is written feature-major [HOR, B_core, QT]; the host
    transposes to [B, HOR, QT] when gathering.
"""

import dataclasses
from contextlib import ExitStack

import numpy as np

import concourse.bass as bass
import concourse.mybir as mybir
import concourse.tile as tile
from concourse import bass_utils
from concourse.bass import ts
from concourse.masks import make_identity

F32 = mybir.dt.float32
BF16 = mybir.dt.bfloat16

B, T, D = 8192, 512, 512
H1, H2, HOR = 256, 128, 96
QF, QT = 7, 32
NCORES = 8
BC = B // NCORES  # batch per core
SUB = 512         # samples per super-tile
GRP = 16          # samples per interp group
GB = 4            # interp groups batched per PSUM bank / A tile
QUANTILE_LEVELS = np.array(
    [0.025, 0.1, 0.25, 0.5, 0.75, 0.9, 0.975], dtype=np.float32
)

# optimal 16-CE sorting network for 7 elements (ascending), disjoint layers
SORT7_LAYERS = [
    [(1, 2), (3, 4), (5, 6)],
    [(0, 2), (3, 5), (4, 6)],
    [(0, 1), (4, 5), (2, 6)],
    [(0, 4), (1, 5)],
    [(0, 3), (2, 5)],
    [(1, 3), (2, 4)],
    [(2, 3)],
]


def _view(ap, free_dims, extra_offset):
    """Rebuild an AP keeping its partition dim, with custom free-dim lattice."""
    dims = [tuple(ap.ap[0])] + [tuple(d) for d in free_dims]
    return dataclasses.replace(ap, ap=tuple(dims), offset=ap.offset + extra_offset)


# ---------------------------------------------------------------------------
# host-side constants / input prep
# ---------------------------------------------------------------------------

def _host_constants(b_eff, b2, b3):
    # bias_all [128, 32]: packed per-partition bias columns
    bias = np.zeros((128, 32), dtype=np.float32)
    for qh in range(QF):
        for mc in range(H1 // 128):
            bias[:, 4 + 2 * qh + mc] = b_eff[qh, 128 * mc : 128 * (mc + 1)]
        bias[:, 18 + qh] = b2[qh]
        bias[:96, 25 + qh] = b3[qh]
    # M112 [112, 512]: block-diagonal 0/1 mask over (sample, coeff) x (sample, t)
    m112 = np.zeros((112, 512), dtype=np.float32)
    for i in range(QF):
        for s in range(GRP):
            m112[GRP * i + s, QT * s : QT * s + QT] = 1.0
    return bias, m112


def _host_coeffs(q_sorted):
    """Interp coefficients for sorted q, in transposed group layout.

    Returns aT packed as [7*GRP, (B//GRP)*QT] fp32 where
    aT[7*sl + i, QT*g + t] = a_i(q_sorted[GRP*g + sl, t]).
    """
    ql = QUANTILE_LEVELS
    v = q_sorted  # [B, QT] fp32
    hi = np.clip(np.searchsorted(ql, v, side="left"), 1, QF - 1)
    lo = hi - 1
    w = ((v - ql[lo]) / (ql[hi] - ql[lo] + np.float32(1e-8))).astype(np.float32)
    a = np.zeros((v.shape[0], QT, QF), dtype=np.float32)
    np.put_along_axis(a, lo[:, :, None], (1.0 - w)[:, :, None], axis=2)
    np.put_along_axis(a, hi[:, :, None], w[:, :, None], axis=2)
    low_mask = v <= ql[0]
    high_mask = v >= ql[-1]
    a[low_mask] = 0.0
    a[high_mask] = 0.0
    a[..., 0] += low_mask.astype(np.float32)
    a[..., QF - 1] += high_mask.astype(np.float32)
    ngrp = v.shape[0] // GRP
    # [B, QT, QF] -> [ngrp, GRP(sl), QT, QF] -> [QF, GRP, ngrp, QT]
    # partition p = GRP*i + sl  (i-major keeps device-side writes contiguous)
    aT = a.reshape(ngrp, GRP, QT, QF).transpose(3, 1, 0, 2)
    return np.ascontiguousarray(aT.reshape(QF * GRP, ngrp * QT))


# ---------------------------------------------------------------------------
# device kernel
# ---------------------------------------------------------------------------

def _emit(ctx: ExitStack, tc: tile.TileContext, ins, outs, bc=BC):
    nc = tc.nc
    xT_d, aT_d, w1_d, w2_d, w3_d, bias_d, m112_d = ins
    (r_d,) = outs
    n_sub = bc // SUB
    ngrp = SUB // GRP

    cpool = ctx.enter_context(tc.tile_pool(name="cpool", bufs=1))
    wpool = ctx.enter_context(tc.tile_pool(name="wpool", bufs=1))
    atpool = ctx.enter_context(tc.tile_pool(name="atpool", bufs=2))
    xTpool = ctx.enter_context(tc.tile_pool(name="xTpool", bufs=2))
    h1pool = ctx.enter_context(tc.tile_pool(name="h1pool", bufs=2))
    h2pool = ctx.enter_context(tc.tile_pool(name="h2pool", bufs=2))
    o3pool = ctx.enter_context(tc.tile_pool(name="o3pool", bufs=12))
    sqgpool = ctx.enter_context(tc.tile_pool(name="sqgpool", bufs=2))
    sqTpool = ctx.enter_context(tc.tile_pool(name="sqTpool", bufs=2))
    apool = ctx.enter_context(tc.tile_pool(name="apool", bufs=2))
    rpool = ctx.enter_context(tc.tile_pool(name="rpool", bufs=4))
    tpsum = ctx.enter_context(tc.tile_pool(name="tpsum", bufs=3, space="PSUM"))
    hpsum = ctx.enter_context(tc.tile_pool(name="hpsum", bufs=2, space="PSUM"))
    rpsum = ctx.enter_context(tc.tile_pool(name="rpsum", bufs=3, space="PSUM"))

    # --- constants ---
    ident = cpool.tile([128, 128], F32)
    make_identity(nc, ident[:])
    bias_sb = cpool.tile([128, 32], F32)
    nc.sync.dma_start(bias_sb[:], bias_d)
    m112 = cpool.tile([112, 512], F32)
    nc.sync.dma_start(m112[:], m112_d)

    # ---- super-tile 0 input loads FIRST so the PE can start as soon as the
    # first W_eff chunks land (weights queue behind on the same DMA queue) ----
    xT_st = []
    aT_st = []
    for st in range(n_sub):
        xT_st.append([None] * 4)
        aT_st.append(None)
    for tci in range(4):
        xt = xTpool.tile([128, SUB], F32, name=f"xT0_{tci}", tag=f"xT{tci}")
        nc.sync.dma_start(xt[:], xT_d[ts(tci, 128), ts(0, SUB)])
        xT_st[0][tci] = xt

    # --- weights (persist across super-tiles); W_eff = W_bb @ W1 host-fused ---
    w1_sb = []
    for qh in range(QF):
        row = []
        for dc in range(D // 128):
            w = wpool.tile([128, H1], F32, name=f"w1_{qh}_{dc}")
            nc.sync.dma_start(w[:], w1_d[qh, ts(dc, 128), :])
            row.append(w)
        w1_sb.append(row)
    aT0 = atpool.tile([112, ngrp * QT], F32, name="aT0", tag="aT")
    nc.sync.dma_start(aT0[:], aT_d[:, ts(0, ngrp * QT)])
    aT_st[0] = aT0
    w2_sb = []
    for qh in range(QF):
        row = []
        for mc in range(H1 // 128):
            w = wpool.tile([128, H2], F32, name=f"w2_{qh}_{mc}")
            nc.sync.dma_start(w[:], w2_d[qh, ts(mc, 128), :])
            row.append(w)
        w2_sb.append(row)
    w3_sb = []
    for qh in range(QF):
        w = wpool.tile([128, HOR], F32, name=f"w3_{qh}")
        nc.sync.dma_start(w[:], w3_d[qh])
        w3_sb.append(w)

    # =====================================================================
    # heads + sort + interpolation, per super-tile of 512 samples
    # =====================================================================
    for st in range(n_sub):
        # ---- loads: transposed x chunks + transposed coefficient block ----
        if xT_st[st][0] is None:
            for tci in range(4):
                xt = xTpool.tile([128, SUB], F32, name=f"xT{st}_{tci}",
                                 tag=f"xT{tci}")
                nc.sync.dma_start(xt[:], xT_d[ts(tci, 128), ts(st, SUB)])
                xT_st[st][tci] = xt
            a_t = atpool.tile([112, ngrp * QT], F32, name=f"aT{st}", tag="aT")
            nc.sync.dma_start(a_t[:], aT_d[:, ts(st, ngrp * QT)])
            aT_st[st] = a_t
        xT = xT_st[st]
        aT = aT_st[st]

        # ---- W_eff: h1T[q][mc] [128, SUB] straight from xT ----
        h1T = [[None] * 2 for _ in range(QF)]
        for qh in range(QF):
            for mc in range(2):
                ps = hpsum.tile([128, SUB], F32, tag="hps")
                for dc in range(4):
                    nc.tensor.matmul(
                        ps[:],
                        lhsT=w1_sb[qh][dc][:, ts(mc, 128)],
                        rhs=xT[dc][:],
                        start=(dc == 0),
                        stop=(dc == 3),
                    )
                h1 = h1pool.tile([128, SUB], F32, name=f"h1_{st}_{qh}_{mc}",
                                  tag=f"h1_{qh}_{mc}", bufs=1)
                nc.scalar.activation(
                    h1[:], ps[:], mybir.ActivationFunctionType.Relu,
                    bias=bias_sb[:, 4 + 2 * qh + mc : 5 + 2 * qh + mc], scale=1.0,
                )
                h1T[qh][mc] = h1

        # ---- W2: h2T[q] [128, SUB] ----
        h2T = [None] * QF
        for qh in range(QF):
            ps = hpsum.tile([128, SUB], F32, tag="hps")
            for mc in range(2):
                nc.tensor.matmul(
                    ps[:], lhsT=w2_sb[qh][mc][:], rhs=h1T[qh][mc][:],
                    start=(mc == 0), stop=(mc == 1),
                )
            h2 = h2pool.tile([128, SUB], F32, name=f"h2_{st}_{qh}", tag=f"h2_{qh}",
                                  bufs=1)
            nc.scalar.activation(
                h2[:], ps[:], mybir.ActivationFunctionType.Relu,
                bias=bias_sb[:, 18 + qh : 19 + qh], scale=1.0,
            )
            h2T[qh] = h2

        # ---- W3: o3[q] [96, SUB] (bias added on DVE) ----
        o3 = [None] * QF
        for qh in range(QF):
            ps = hpsum.tile([HOR, SUB], F32, tag="hps")
            nc.tensor.matmul(
                ps[:], lhsT=w3_sb[qh][:, :], rhs=h2T[qh][:],
                start=True, stop=True,
            )
            o = o3pool.tile([HOR, SUB], F32, name=f"o3_{st}_{qh}", tag="sortt")
            nc.scalar.activation(
                o[:], ps[:], mybir.ActivationFunctionType.Identity,
                bias=bias_sb[:HOR, 25 + qh : 26 + qh], scale=1.0,
            )
            o3[qh] = o

        # ---- sort the 7 head outputs elementwise on DVE with flat
        # contiguous APs (3-D views run the DVE slow path); final values
        # written interleaved into SQG (free idx = 112*g + 16*i + s).
        # The sort runs in two sample-halves with separate SQG tiles so the
        # interp phase can start as soon as the first half is sorted.
        last_touch = {}
        for li, layer in enumerate(SORT7_LAYERS):
            for (a, b) in layer:
                last_touch[a] = (li, a, b)
                last_touch[b] = (li, a, b)

        HGRP = ngrp // 2   # groups per half
        HSUB = SUB // 2    # samples per half
        sqgs = []
        for hf in range(2):
            sqg = sqgpool.tile([HOR, HGRP * 112], F32, name=f"sqg{st}_{hf}",
                               tag=f"sqg{hf}")
            sqgs.append(sqg)
            c0 = HSUB * hf

            def sqg_slot(j):
                return _view(sqg[:], [(112, HGRP), (1, GRP)], GRP * j)

            cur = {k: o3[k] for k in range(QF)}
            ce_idx = 0
            for li, layer in enumerate(SORT7_LAYERS):
                for (a, b) in layer:
                    a_final = last_touch[a] == (li, a, b)
                    b_final = last_touch[b] == (li, a, b)
                    def flat(t):
                        return t[:, c0 : c0 + HSUB] if t[:].shape[1] == SUB \
                            else t[:]
                    def grouped(t):
                        return flat(t).rearrange("p (g s) -> p g s", g=HGRP)
                    if a_final:
                        oa, ia, ib_a = sqg_slot(a), grouped(cur[a]), grouped(cur[b])
                    else:
                        ta = o3pool.tile([HOR, HSUB], F32,
                                         name=f"s{st}_{hf}_{ce_idx}a",
                                         tag="sorth")
                        oa, ia, ib_a = ta[:], flat(cur[a]), flat(cur[b])
                    if b_final:
                        ob, ia_b, ib_b = sqg_slot(b), grouped(cur[a]), grouped(cur[b])
                    else:
                        tb = o3pool.tile([HOR, HSUB], F32,
                                         name=f"s{st}_{hf}_{ce_idx}b",
                                         tag="sorth")
                        ob, ia_b, ib_b = tb[:], flat(cur[a]), flat(cur[b])
                    nc.vector.tensor_tensor(oa, ia, ib_a, op=mybir.AluOpType.min)
                    nc.vector.tensor_tensor(ob, ia_b, ib_b, op=mybir.AluOpType.max)
                    if not a_final:
                        cur[a] = ta
                    if not b_final:
                        cur[b] = tb
                    ce_idx += 1

        # ---- per 4-group batch: transposes into one PSUM bank + single
        # evacuation, batched A build, then per-group interp matmul ----
        for bi in range(ngrp // GB):
            ps_t = tpsum.tile([112, GB * HOR], F32, tag="tps")
            for j in range(GB):
                g = GB * bi + j
                hf, gl = divmod(g, HGRP)
                nc.tensor.transpose(
                    ps_t[:, ts(j, HOR)],
                    sqgs[hf][:, 112 * gl : 112 * (gl + 1)],
                    ident[:HOR, :HOR],
                )
            sqa = sqTpool.tile([112, GB * HOR], F32, tag="sqa")
            nc.vector.tensor_copy(sqa[:], ps_t[:])

            # A [112, GB*512] = broadcast(aT 4-group block) * M112
            A = apool.tile([112, GB * 512], F32, tag="A")
            av = aT[:, GB * QT * bi : GB * QT * (bi + 1)].rearrange(
                "p (j t) -> p j t", j=GB
            ).unsqueeze(2).broadcast_to((112, GB, GRP, QT))
            mv = m112[:].rearrange("p (s t) -> p s t", s=GRP).unsqueeze(
                1
            ).broadcast_to((112, GB, GRP, QT))
            Av = A[:].rearrange("p (j s t) -> p j s t", j=GB, s=GRP)
            nc.vector.tensor_tensor(Av, av, mv, op=mybir.AluOpType.mult)

            for j in range(GB):
                g = GB * bi + j
                gg = st * ngrp + g  # global group index
                rps = rpsum.tile([HOR, 512], F32, tag="rps")
                nc.tensor.matmul(
                    rps[:], lhsT=sqa[:, ts(j, HOR)],
                    rhs=A[:, ts(j, 512)], start=True, stop=True,
                )
                r_sb = rpool.tile([HOR, 512], BF16, tag="rsb")
                if g % 2 == 0:
                    nc.scalar.copy(r_sb[:], rps[:])
                else:
                    nc.vector.tensor_copy(r_sb[:], rps[:])
                nc.scalar.dma_start(
                    r_d[:, GRP * gg : GRP * (gg + 1), :],
                    r_sb[:].rearrange("p (s t) -> p s t", s=GRP),
                )


# Per-instruction-type sync-wait slot capacity in the walrus ISA descriptors.
_WAIT_CAPACITY = {}  # default: every type gets a single wait slot
_DRAIN_CAPACITY = {
    "EngineType.SP": 1,
    "EngineType.PE": 1,
}


def _split_waits(nc):
    """Some walrus ISA descriptors (LDWEIGHTS, DMA) have too few sync-wait
    slots for the waits Tile emits.  Move surplus waits of overflowing
    instructions onto drains inserted right before them on the same queue."""
    for fn in nc.m.functions:
        for blk in fn.blocks:
            insts = list(blk.instructions)
            out = []
            changed = False
            for ins in insts:
                si = ins.sync_info
                cap = _WAIT_CAPACITY.get(type(ins).__name__, 1)
                if si is not None and si.on_wait and len(si.on_wait) > cap:
                    waits = list(si.on_wait)
                    surplus = waits[:-cap]
                    dcap = _DRAIN_CAPACITY.get(str(ins.engine), 1)
                    di = 0
                    while surplus:
                        chunk, surplus = surplus[:dcap], surplus[dcap:]
                        out.append(
                            mybir.InstDrain(
                                name=f"{ins.name}-wfence{di}",
                                engine=ins.engine,
                                ins=[],
                                outs=[],
                                sync_info=mybir.SyncInfo(
                                    on_wait=chunk, on_update=[]
                                ),
                            )
                        )
                        di += 1
                    si.on_wait = waits[-cap:]
                    changed = True
                out.append(ins)
            if changed:
                blk.instructions = out


def build_module(bc=BC):
    nc = bass.Bass("TRN2", target_bir_lowering=False, debug=False)
    xT_d = nc.dram_tensor("xT", [T, bc], F32, kind="ExternalInput").ap()
    aT_d = nc.dram_tensor("aT", [GRP * QF, (bc // GRP) * QT], F32,
                          kind="ExternalInput").ap()
    w1_d = nc.dram_tensor("Weff", [QF, T, H1], F32, kind="ExternalInput").ap()
    w2_d = nc.dram_tensor("W2", [QF, H1, H2], F32, kind="ExternalInput").ap()
    w3_d = nc.dram_tensor("W3", [QF, H2, HOR], F32, kind="ExternalInput").ap()
    bias_d = nc.dram_tensor("bias_all", [128, 32], F32, kind="ExternalInput").ap()
    m112_d = nc.dram_tensor("m112", [112, 512], F32, kind="ExternalInput").ap()
    r_d = nc.dram_tensor("r_out", [HOR, bc, QT], BF16, kind="ExternalOutput").ap()

    with tile.TileContext(nc) as tc:
        with ExitStack() as ctx:
            _emit(ctx, tc, (xT_d, aT_d, w1_d, w2_d, w3_d, bias_d, m112_d),
                  (r_d,), bc=bc)
    _split_waits(nc)
    return nc


_NC_CACHE = {}
LAST_RESULTS = None


def kernel(**inputs) -> np.ndarray:
    global LAST_RESULTS
    x = np.asarray(inputs["x"], dtype=np.float32)
    q = np.asarray(inputs["q"], dtype=np.float32)
    w_bb = np.asarray(inputs["W_bb"], dtype=np.float64)
    w1_64 = np.asarray(inputs["W1"], dtype=np.float64)
    # Fold the (linear, dropout-free) backbone into the first head layer:
    # h1 = relu(x @ (W_bb @ W1[q]) + (b_bb @ W1[q] + b1[q])).
    w_eff = np.ascontiguousarray(
        np.einsum("td,qdk->qtk", w_bb, w1_64).astype(np.float32)
    )
    b_eff = (
        np.asarray(inputs["b_bb"], dtype=np.float64) @ w1_64
        + np.asarray(inputs["b1"], dtype=np.float64)
    ).astype(np.float32)
    w2 = np.ascontiguousarray(np.asarray(inputs["W2"], dtype=np.float32))
    w3 = np.ascontiguousarray(np.asarray(inputs["W3"], dtype=np.float32))
    bias, m112 = _host_constants(
        b_eff,
        np.asarray(inputs["b2"], dtype=np.float32),
        np.asarray(inputs["b3"], dtype=np.float32),
    )
    q_sorted = np.sort(q, axis=1)

    if BC not in _NC_CACHE:
        _NC_CACHE[BC] = build_module(BC)
    nc = _NC_CACHE[BC]

    in_maps = []
    for c in range(NCORES):
        sl = slice(BC * c, BC * (c + 1))
        in_maps.append(
            {
                "xT": np.ascontiguousarray(x[sl].T),
                "aT": _host_coeffs(q_sorted[sl]),
                "Weff": w_eff,
                "W2": w2,
                "W3": w3,
                "bias_all": bias,
                "m112": m112,
            }
        )

    res = bass_utils.run_bass_kernel_spmd(nc, in_maps, core_ids=list(range(NCORES)))
    LAST_RESULTS = res
    out = np.empty((B, HOR, QT), dtype=np.float32)
    for c in range(NCORES):
        out[BC * c : BC * (c + 1)] = np.transpose(
            res.results[c]["r_out"].astype(np.float32), (1, 0, 2)
        )
    return out
